# revision 32
# baseline (speedup 1.0000x reference)
"""AttentionBlock Trainium2 kernel (self-contained).

Problem: x[4,256,64,64] -> qkv 1x1 conv -> single-head self-attention over
the 4096 spatial tokens -> out 1x1 conv -> residual.

Under the axon relay the end-to-end time of kernel() is dominated by
host<->device transfer over the tunnel (~56 MB/s up, ~30 MB/s down measured),
not on-device compute, so the design minimizes shipped bytes:

  - x ships once per core in fp8 (e4m3): the attention path tolerates fp8
    activation noise, and the f32 residual (x + o) is added on the HOST, so
    the device never needs a precise copy of x.
  - The q/k weights are folded on the host: the k-side bias cancels in
    softmax and scores only need t = A x_q + c with A = W_q^T W_k (256x256)
    and c = W_k^T b_q, so W_q/W_k never ship -- just A (bf16). Together with
    W_v^T and W_o^T, all weights go in ONE packed bf16 tensor [256, 768].
  - Output is the projected o WITHOUT the residual, in bf16 (o is O(+-5), so
    bf16 keeps max error ~2e-3 of the output absmax); the host adds x in f32.

Sharding: 8 cores = 4 batch elements x 2 query halves. Each core handles one
batch element's full K/V token range (4096) and 2048 queries, flash-style
on-chip: the [4096 x 2048] score matrix never touches HBM. The query half is
pre-rotated to columns 0:2047 of the per-core x so a single SPMD program
serves both halves (key order is irrelevant inside the softmax sum).

Per-core dataflow (feature-major x8 = rot(x[b]) reshaped [256, 4096], fp8):
  - upconvert x8 -> bf16 once in SBUF; all matmuls run bf16 with f32 PSUM.
  - t = A x_q + c  [256 x 2048] (ACT adds c via per-partition bias).
  - v = W_v x + b_v token-major [tok, e] with a ones-column appended, so the
    softmax normalizer Z drops out of the attn@v matmul for free.
  - Scores k-major: S^T[k, q] = x^T t; exp via ACT (scale=1/sqrt(E), exact
    fp32 PSUM in, bf16 out), directly the stationary operand of attn@v.
  - Softmax without max-subtraction: scores are O(+-7) for unit-scale data.
  - attn@v gives o token-major [q, e] plus Z in column 256; normalize by
    1/Z per-partition, PE-transpose 128x128 blocks to feature-major, then
    out-projection + bias per q block, DMA out in bf16.

Measured rel err vs the fp32 reference: ~2e-3 absmax-relative (fp8 x noise
through the attention path; the residual is exact f32 host-side).
"""

import contextlib

import ml_dtypes
import numpy as np

import jax
from jax.experimental.shard_map import shard_map
from jax.sharding import Mesh, NamedSharding, PartitionSpec

import concourse.bacc as bacc
import concourse.bass2jax as bass2jax
import concourse.tile as tile
from concourse import mybir
from concourse.bass_utils import run_bass_kernel_spmd

F32 = mybir.dt.float32
BF16 = mybir.dt.bfloat16
FP8 = mybir.dt.float8e4
INT8 = mybir.dt.int8
AF = mybir.ActivationFunctionType
AX = mybir.AxisListType
ALU = mybir.AluOpType

# ---------------------------------------------------------------------------
# run_bass_via_pjrt rebuilds jax.jit(shard_map(...)) from a fresh closure on
# every call, so each kernel() invocation pays a full retrace + relower
# (~120ms). Patch in a behaviorally identical variant that caches the jitted
# callable per (nc, n_cores); run_bass_kernel_spmd picks it up via the module
# attribute. Every call still ships all inputs and executes on hardware.
_ORIG_RUN_VIA_PJRT = bass2jax.run_bass_via_pjrt
_JIT_CACHE = {}
# ncs whose programs write every element of every output: for these the
# pre-zeroed output operands can be cached device-resident (no donation, no
# per-call host->device zeros upload) -- the NEFF output never depends on
# their initial contents.
_WRITES_ALL_OUTPUTS = set()


def _cached_run_bass_via_pjrt(nc, in_maps, n_cores):
    if (nc.dbg_addr is not None or n_cores == 1
            or id(nc) not in _WRITES_ALL_OUTPUTS):
        return _ORIG_RUN_VIA_PJRT(nc, in_maps, n_cores)
    key = (id(nc), n_cores)
    ent = _JIT_CACHE.get(key)
    if ent is None:
        bass2jax.install_neuronx_cc_hook()
        partition_name = (nc.partition_id_tensor.name
                          if nc.partition_id_tensor else None)
        in_names, out_names, out_avals = [], [], []
        for alloc in nc.m.functions[0].allocations:
            if not isinstance(alloc, mybir.MemoryLocationSet):
                continue
            name = alloc.memorylocations[0].name
            if alloc.kind == "ExternalInput":
                if name != partition_name:
                    in_names.append(name)
            elif alloc.kind == "ExternalOutput":
                out_names.append(name)
                out_avals.append(jax.core.ShapedArray(
                    tuple(alloc.tensor_shape), mybir.dt.np(alloc.dtype)))
        n_params, n_outs = len(in_names), len(out_names)
        all_in = tuple(in_names + out_names
                       + ([partition_name] if partition_name else []))

        def _body(*args):
            operands = list(args)
            if partition_name is not None:
                operands.append(bass2jax.partition_id_tensor())
            outs = bass2jax._bass_exec_p.bind(
                *operands, out_avals=tuple(out_avals), in_names=all_in,
                out_names=tuple(out_names), lowering_input_output_aliases=(),
                sim_require_finite=True, sim_require_nnan=True, nc=nc)
            return tuple(outs)

        mesh = Mesh(np.asarray(jax.devices()[:n_cores]), ("core",))
        sharded = jax.jit(
            shard_map(_body, mesh=mesh,
                      in_specs=(PartitionSpec("core"),) * (n_params + n_outs),
                      out_specs=(PartitionSpec("core"),) * n_outs,
                      check_rep=False),
            keep_unused=True)
        sh = NamedSharding(mesh, PartitionSpec("core"))
        zdev = [jax.device_put(
            np.zeros((n_cores * av.shape[0], *av.shape[1:]), av.dtype), sh)
            for av in out_avals]
        ent = (sharded, in_names, out_names, out_avals, zdev)
        _JIT_CACHE[key] = ent
    sharded, in_names, out_names, out_avals, zdev = ent
    override = getattr(nc, "_concat_override", None)
    if override is not None:
        concat_in = [override[nm] for nm in in_names]
    else:
        concat_in = [
            np.concatenate([np.asarray(m[nm]) for m in in_maps], axis=0)
            for nm in in_names]
    out_arrs = sharded(*concat_in, *zdev)
    # kick off all shard downloads concurrently before the blocking reads
    for o in out_arrs:
        for s in o.addressable_shards:
            s.data.copy_to_host_async()
    return [
        {nm: np.asarray(out_arrs[i]).reshape(n_cores, *out_avals[i].shape)[c]
         for i, nm in enumerate(out_names)}
        for c in range(n_cores)
    ]


bass2jax.run_bass_via_pjrt = _cached_run_bass_via_pjrt

E = 256          # embed dim
NTOK = 4096      # tokens per batch element (64*64)
NQ = 2048        # queries per core
P = 128          # partitions
NEC = 2          # e-chunks (E / P)
NKC = NTOK // P  # 32 k-chunks
QB = 512         # q block (scores free dim)
NQB = NQ // QB   # q blocks
EXP_SCALE = 1.0 / 16.0  # 1/sqrt(E)

NP_FP8 = ml_dtypes.float8_e4m3
NP_BF16 = ml_dtypes.bfloat16

N_CORES = 8


def build_nc(reps=1):
    """reps != 1 wraps the body in a HW For_i loop (used only for wall-clock
    timing via the reps-slope method; the production path is reps=1)."""
    nc = bacc.Bacc(None, target_bir_lowering=False, num_devices=N_CORES)

    # each core ships only its own query half; the peer's half (needed for
    # K/V) arrives device-to-device via a pair AllGather
    xh = nc.dram_tensor("xh", [E, NQ], FP8, kind="ExternalInput")
    # packed weights [256, 832]: [0:E]=A (t = A x_q stationary), [E:2E]=W_v^T,
    # [2E:3E]=W_o^T, [3E:3E+64]=identity (two [128,64] halves stacked).
    # Each core ships only a 1/8 column shard; an 8-rank AllGather rebuilds it.
    WCOLS = 3 * E + 64
    WSH = WCOLS // N_CORES
    wsh = nc.dram_tensor("wsh", [E, WSH], BF16, kind="ExternalInput")
    # packed per-partition biases: cols 0:2 = c (t bias), 2:4 = out bias
    bpk = nc.dram_tensor("bpk", [P, 4], F32, kind="ExternalInput")
    bv = nc.dram_tensor("bv", [E], F32, kind="ExternalInput")
    # o ships int8 with a per-(feature-row, 256-query-block) scale: same
    # precision class as bf16 at half the tunnel bytes
    out = nc.dram_tensor("out", [E, NQ], INT8, kind="ExternalOutput")
    osc = nc.dram_tensor("osc", [P, NEC, 8], F32, kind="ExternalOutput")

    with tile.TileContext(nc) as tc:
        with (
            tc.tile_pool(name="dram", bufs=1, space="DRAM") as dramp,
            tc.tile_pool(name="const", bufs=1) as const,
            tc.tile_pool(name="x8p", bufs=1) as x8p,
            tc.tile_pool(name="xpool", bufs=1) as xpool,
            tc.tile_pool(name="kqv", bufs=1) as kqv,
            tc.tile_pool(name="expp", bufs=2) as expp,
            tc.tile_pool(name="ofm", bufs=1) as ofm,
            tc.tile_pool(name="small", bufs=4) as small,
            tc.tile_pool(name="outp", bufs=3) as outp,
            tc.tile_pool(name="oscp", bufs=1) as oscp,
            tc.tile_pool(name="psA", bufs=2, space="PSUM") as psA,
            tc.tile_pool(name="psO", bufs=2, space="PSUM") as psO,
            tc.tile_pool(name="psT", bufs=2, space="PSUM") as psT,
        ):
            t = {}

            def emit_loads():
                # weights: 8-rank AllGather of the 1/8 column shards
                wg_in = dramp.tile([E, WSH], BF16, tag="wg_in", name="wg_in")
                wg = dramp.tile([N_CORES, E, WSH], BF16, tag="wg",
                                addr_space="Shared", name="wg")
                nc.sync.dma_start(out=wg_in[:, :], in_=wsh[:, :])
                nc.gpsimd.collective_compute(
                    "AllGather", ALU.bypass,
                    replica_groups=[list(range(N_CORES))],
                    ins=[wg_in[:, :]], outs=[wg[:, :, :]])
                t["wpk_sb"] = const.tile([P, NEC, WCOLS], BF16, tag="wpk",
                                         name="wpk_sb")
                for r in range(N_CORES):
                    for ec in range(NEC):
                        nc.sync.dma_start(
                            out=t["wpk_sb"][:, ec, r * WSH:(r + 1) * WSH],
                            in_=wg[r, ec * P:(ec + 1) * P, :])
                t["bpk_sb"] = const.tile([P, 4], F32, tag="bpk", name="bpk_sb")
                nc.sync.dma_start(out=t["bpk_sb"], in_=bpk[:, :])
                t["bv_bc"] = const.tile([P, E], F32, tag="bv", name="bv_bc")
                nc.sync.dma_start(out=t["bv_bc"],
                                  in_=bv[:].partition_broadcast(P))
                # identity for PE transpose: unpack the two [128, 64] halves
                t["ident_sb"] = const.tile([P, P], BF16, tag="ident",
                                           name="ident_sb")
                for ec in range(NEC):
                    nc.vector.tensor_copy(
                        t["ident_sb"][:, ec * 64:(ec + 1) * 64],
                        t["wpk_sb"][:, ec, 3 * E:3 * E + 64])
                # pair AllGather: own query half -> both halves of x[b].
                # xg is flat rank-concat, so declare it [2, E, NQ].
                xg_in = dramp.tile([E, NQ], FP8, tag="xg_in", name="xg_in")
                xg = dramp.tile([2, E, NQ], FP8, tag="xg", name="xg")
                nc.sync.dma_start(out=xg_in[:, :], in_=xh[:, :])
                nc.gpsimd.collective_compute(
                    "AllGather", ALU.bypass,
                    replica_groups=[[2 * i, 2 * i + 1]
                                    for i in range(N_CORES // 2)],
                    ins=[xg_in[:, :]], outs=[xg[:, :, :]])
                # query path straight from the input (position-independent
                # of core parity); not gated on the collective
                t["xq8_sb"] = x8p.tile([P, NEC, NQ], FP8, tag="xq8",
                                       name="xq8_sb")
                t["xq_sb"] = xpool.tile([P, NEC, NQ], BF16, tag="xq",
                                        name="xq_sb")
                for tt in range(NQ // 512):
                    for ec in range(NEC):
                        nc.sync.dma_start(
                            out=t["xq8_sb"][:, ec, tt * 512:(tt + 1) * 512],
                            in_=xh[ec * P:(ec + 1) * P,
                                   tt * 512:(tt + 1) * 512])
                for tt in range(NQ // 512):
                    for ec in range(NEC):
                        nc.vector.tensor_copy(
                            t["xq_sb"][:, ec, tt * 512:(tt + 1) * 512],
                            t["xq8_sb"][:, ec, tt * 512:(tt + 1) * 512])
                # K/V token range: both halves from the gathered buffer
                t["x8_sb"] = x8p.tile([P, NEC, NTOK], FP8, tag="x8",
                                      name="x8_sb")
                t["xb_sb"] = xpool.tile([P, NEC, NTOK], BF16, tag="xb",
                                        name="xb_sb")
                for r in range(2):
                    for tt in range(NQ // 512):
                        for ec in range(NEC):
                            nc.sync.dma_start(
                                out=t["x8_sb"][:, ec,
                                               r * NQ + tt * 512:
                                               r * NQ + (tt + 1) * 512],
                                in_=xg[r, ec * P:(ec + 1) * P,
                                       tt * 512:(tt + 1) * 512])
                for tt in range(NTOK // 512):
                    for ec in range(NEC):
                        nc.vector.tensor_copy(
                            t["xb_sb"][:, ec, tt * 512:(tt + 1) * 512],
                            t["x8_sb"][:, ec, tt * 512:(tt + 1) * 512])

            def emit_compute():
                wpk_sb, bpk_sb = t["wpk_sb"], t["bpk_sb"]
                bv_bc, ident_sb, xb_sb = t["bv_bc"], t["ident_sb"], t["xb_sb"]
                xq_sb = t["xq_sb"]

                t_sb = kqv.tile([P, NEC, NQ], BF16, tag="t", name="t_sb")
                v_sb = kqv.tile([P, NKC, E + 1], BF16, tag="v", name="v_sb")

                # ---- t = A x_q + c  (A = W_q^T W_k folded host-side; the
                # k-bias cancels in softmax and q/k are never materialized)
                for tt in range(NQ // 512):
                    for eo in range(NEC):
                        ps_full = psA.tile([P, 2, QB], F32, tag="sc",
                                           name="ps_t")
                        ps = ps_full[:, 0, :]
                        for ec in range(NEC):
                            nc.tensor.matmul(
                                ps,
                                wpk_sb[:, ec, eo * P:(eo + 1) * P],
                                xq_sb[:, ec, tt * 512:(tt + 1) * 512],
                                start=(ec == 0), stop=(ec == NEC - 1))
                        nc.scalar.activation(
                            t_sb[:, eo, tt * 512:(tt + 1) * 512], ps,
                            AF.Identity, bias=bpk_sb[:, eo:eo + 1])

                # ---- v = W_v x + b_v, token-major, ones column for Z
                for tcb in range(NKC):
                    ps_full = psA.tile([P, 2, QB], F32, tag="sc", name="ps_v")
                    ps = ps_full[:, 0, :E]
                    for ec in range(NEC):
                        nc.tensor.matmul(
                            ps,
                            xb_sb[:, ec, tcb * P:(tcb + 1) * P],
                            wpk_sb[:, ec, E:2 * E],
                            start=(ec == 0), stop=(ec == NEC - 1))
                    nc.vector.tensor_add(v_sb[:, tcb, 0:E], ps, bv_bc)
                nc.vector.memset(v_sb[:, :, E:E + 1], 1.0)

                o_fm = ofm.tile([P, NEC, NQ], BF16, tag="o_fm", name="o_fm")
                osc_sb = oscp.tile([P, NEC, 8], F32, tag="osc", name="osc_sb")

                # ---- attention, per q block
                for qb in range(NQB):
                    q0 = qb * QB
                    expS = expp.tile([P, NKC, QB], BF16, tag="expS",
                                     name="expS")
                    for kcg in range(NKC // 2):
                        ps = psA.tile([P, 2, QB], F32, tag="sc", name="ps_s")
                        for kk in range(2):
                            kc = kcg * 2 + kk
                            for ec in range(NEC):
                                nc.tensor.matmul(
                                    ps[:, kk, :],
                                    xb_sb[:, ec, kc * P:(kc + 1) * P],
                                    t_sb[:, ec, q0:q0 + QB],
                                    start=(ec == 0), stop=(ec == NEC - 1))
                        nc.scalar.activation(
                            expS[:, kcg * 2:(kcg + 1) * 2, :], ps, AF.Exp,
                            scale=EXP_SCALE)
                    for qq in range(QB // P):
                        po = psO.tile([P, E + 1], F32, tag="po", name="po")
                        for kc in range(NKC):
                            nc.tensor.matmul(
                                po,
                                expS[:, kc, qq * P:(qq + 1) * P],
                                v_sb[:, kc, :],
                                start=(kc == 0), stop=(kc == NKC - 1))
                        zr = small.tile([P, 1], F32, tag="zr", name="zr")
                        nc.vector.reciprocal(zr, po[:, E:E + 1])
                        o_tm = small.tile([P, E], BF16, tag="o_tm",
                                          name="o_tm")
                        nc.vector.tensor_scalar_mul(o_tm, po[:, 0:E], zr)
                        for ec in range(NEC):
                            pt = psT.tile([P, P], BF16, tag="pt", name="pt")
                            nc.tensor.transpose(
                                pt, o_tm[:, ec * P:(ec + 1) * P], ident_sb)
                            nc.vector.tensor_copy(
                                o_fm[:, ec, q0 + qq * P:q0 + (qq + 1) * P], pt)

                    # out projection + bias for this q block (residual is
                    # added host-side in f32); quantize to int8 with a
                    # per-(row, block) scale = absmax/127
                    for fc in range(NEC):
                        for qh in range(QB // 256):
                            blk = qb * 2 + qh
                            pso = psO.tile([P, E + 1], F32, tag="po",
                                           name="pso")
                            ps2 = pso[:, 0:256]
                            for ec in range(NEC):
                                nc.tensor.matmul(
                                    ps2,
                                    wpk_sb[:, ec,
                                           2 * E + fc * P:2 * E + (fc + 1) * P],
                                    o_fm[:, ec,
                                         q0 + qh * 256:q0 + (qh + 1) * 256],
                                    start=(ec == 0), stop=(ec == NEC - 1))
                            t2 = outp.tile([P, 256], F32, tag="t2", name="t2")
                            nc.vector.tensor_scalar_add(
                                t2, ps2, bpk_sb[:, 2 + fc:3 + fc])
                            rmax = small.tile([P, 1], F32, tag="rmax",
                                              name="rmax")
                            nc.vector.tensor_reduce(
                                rmax, t2, axis=AX.X, op=ALU.max,
                                apply_absolute_value=True)
                            nc.scalar.activation(
                                osc_sb[:, fc, blk:blk + 1], rmax,
                                AF.Identity, scale=1.0 / 127.0)
                            inv = small.tile([P, 1], F32, tag="inv",
                                             name="inv")
                            nc.vector.reciprocal(
                                inv, osc_sb[:, fc, blk:blk + 1])
                            t8 = outp.tile([P, 256], INT8, tag="t8",
                                           name="t8")
                            nc.vector.tensor_scalar_mul(t8, t2, inv)
                            nc.sync.dma_start(
                                out=out[fc * P:(fc + 1) * P,
                                        q0 + qh * 256:q0 + (qh + 1) * 256],
                                in_=t8)
                nc.sync.dma_start(out=osc[:, :, :], in_=osc_sb[:, :, :])

            loop_ctx = (tc.For_i(0, reps, 1) if reps != 1
                        else contextlib.nullcontext())
            with loop_ctx:
                emit_loads()
                emit_compute()

    nc.compile()
    return nc


_NC = {}


def _get_nc(reps=1):
    if reps not in _NC:
        _NC[reps] = build_nc(reps)
        if reps != 0:
            # the program writes every element of out/osc, so the cached
            # device-resident zero operands path is safe
            _WRITES_ALL_OUTPUTS.add(id(_NC[reps]))
    return _NC[reps]


def make_in_maps(x, qkv_w, qkv_b, out_w, out_b):
    b, e, h, w = x.shape
    n = h * w
    qkv_w = np.asarray(qkv_w).astype(np.float32)
    qkv_b = np.asarray(qkv_b).astype(np.float32)
    out_w = np.asarray(out_w).astype(np.float32)
    out_b = np.asarray(out_b).astype(np.float32)
    xf = np.ascontiguousarray(np.asarray(x, dtype=np.float32).reshape(b, e, n))
    x8 = xf.astype(NP_FP8)
    wq, wk, wv = qkv_w[:E], qkv_w[E:2 * E], qkv_w[2 * E:]
    # t = A x_q + c reproduces q.k up to the k-bias (constant over keys,
    # cancels in softmax): stationary A[e2, e1] = sum_f wq[f,e2] wk[f,e1]
    A = wq.T @ wk
    c = wk.T @ qkv_b[:E]
    ident = np.eye(P, dtype=np.float32)
    ident_pack = np.concatenate([ident[:, :64], ident[:, 64:]], axis=0)
    wpk = np.ascontiguousarray(np.concatenate(
        [A, wv.T, out_w.T, ident_pack], axis=1)).astype(NP_BF16)
    WSH = wpk.shape[1] // N_CORES
    bpk = np.ascontiguousarray(
        np.stack([c[:P], c[P:], out_b[:P], out_b[P:]], axis=1)
    ).astype(np.float32)
    bv_a = np.ascontiguousarray(qkv_b[2 * E:])
    in_maps = []
    for core in range(N_CORES):
        bi, half = divmod(core, 2)
        in_maps.append({
            "xh": np.ascontiguousarray(x8[bi][:, half * NQ:(half + 1) * NQ]),
            "wsh": np.ascontiguousarray(wpk[:, core * WSH:(core + 1) * WSH]),
            "bpk": bpk, "bv": bv_a,
        })
    return in_maps


def assemble(results, x, xf=None):
    b, e, h, w = x.shape
    n = h * w
    if xf is None:
        xf = np.asarray(x, dtype=np.float32).reshape(b, e, n)
    out = np.empty((b, e, n), np.float32)
    for core in range(N_CORES):
        bi, half = divmod(core, 2)
        sl = slice(half * NQ, (half + 1) * NQ)
        o = np.asarray(results[core]["out"]).astype(np.float32)
        sc = np.asarray(results[core]["osc"])  # [P, NEC, 8] = rows p, fc, blk
        o = o.reshape(NEC, P, 8, 256) * sc.transpose(1, 0, 2)[:, :, :, None]
        out[bi][:, sl] = xf[bi][:, sl] + o.reshape(E, NQ)
    return out.reshape(b, e, h, w)


def kernel(x, qkv_w, qkv_b, out_w, out_b):
    b, e, h, w = x.shape
    n = h * w
    qkv_w = np.asarray(qkv_w).astype(np.float32)
    qkv_b = np.asarray(qkv_b).astype(np.float32)
    out_w = np.asarray(out_w).astype(np.float32)
    out_b = np.asarray(out_b).astype(np.float32)
    xf = np.asarray(x, dtype=np.float32).reshape(b, e, n)
    # build the per-core inputs directly in their concatenated global layout
    # (the patched run path consumes these without re-copying)
    x_g = np.empty((N_CORES * E, NQ), NP_FP8)
    for core in range(N_CORES):
        bi, half = divmod(core, 2)
        x_g[core * E:(core + 1) * E] = (
            xf[bi][:, half * NQ:(half + 1) * NQ].astype(NP_FP8))
    wq, wk, wv = qkv_w[:E], qkv_w[E:2 * E], qkv_w[2 * E:]
    A = wq.T @ wk
    c = wk.T @ qkv_b[:E]
    ident = np.eye(P, dtype=np.float32)
    ident_pack = np.concatenate([ident[:, :64], ident[:, 64:]], axis=0)
    wpk = np.concatenate([A, wv.T, out_w.T, ident_pack], axis=1).astype(
        NP_BF16)
    WSH = wpk.shape[1] // N_CORES
    wsh_g = np.ascontiguousarray(
        wpk.reshape(E, N_CORES, WSH).transpose(1, 0, 2).reshape(
            N_CORES * E, WSH))
    bpk = np.stack([c[:P], c[P:], out_b[:P], out_b[P:]], axis=1).astype(
        np.float32)
    bpk_g = np.tile(bpk, (N_CORES, 1))
    bv_g = np.tile(np.ascontiguousarray(qkv_b[2 * E:]), N_CORES)
    in_maps = [
        {"xh": x_g[core * E:(core + 1) * E],
         "wsh": wsh_g[core * E:(core + 1) * E],
         "bpk": bpk_g[core * P:(core + 1) * P],
         "bv": bv_g[core * E:(core + 1) * E]}
        for core in range(N_CORES)
    ]
    nc = _get_nc()
    nc._concat_override = {"xh": x_g, "wsh": wsh_g, "bpk": bpk_g, "bv": bv_g}
    try:
        res = run_bass_kernel_spmd(nc, in_maps, core_ids=list(range(N_CORES)))
    finally:
        nc._concat_override = None
    return assemble(res.results, x, xf=xf)


# revision 34
# speedup vs baseline: 1.0705x; 1.0705x over previous
"""AttentionBlock Trainium2 kernel (self-contained).

Problem: x[4,256,64,64] -> qkv 1x1 conv -> single-head self-attention over
the 4096 spatial tokens -> out 1x1 conv -> residual.

Under the axon relay the end-to-end time of kernel() is dominated by
host<->device transfer over the tunnel (~56 MB/s up, ~30 MB/s down measured),
not on-device compute, so the design minimizes shipped bytes:

  - x ships once per core in fp8 (e4m3): the attention path tolerates fp8
    activation noise, and the f32 residual (x + o) is added on the HOST, so
    the device never needs a precise copy of x.
  - The q/k weights are folded on the host: the k-side bias cancels in
    softmax and scores only need t = A x_q + c with A = W_q^T W_k (256x256)
    and c = W_k^T b_q, so W_q/W_k never ship -- just A (bf16). Together with
    W_v^T and W_o^T, all weights go in ONE packed bf16 tensor [256, 768].
  - Output is the projected o WITHOUT the residual, in bf16 (o is O(+-5), so
    bf16 keeps max error ~2e-3 of the output absmax); the host adds x in f32.

Sharding: 8 cores = 4 batch elements x 2 query halves. Each core handles one
batch element's full K/V token range (4096) and 2048 queries, flash-style
on-chip: the [4096 x 2048] score matrix never touches HBM. The query half is
pre-rotated to columns 0:2047 of the per-core x so a single SPMD program
serves both halves (key order is irrelevant inside the softmax sum).

Per-core dataflow (feature-major x8 = rot(x[b]) reshaped [256, 4096], fp8):
  - upconvert x8 -> bf16 once in SBUF; all matmuls run bf16 with f32 PSUM.
  - t = A x_q + c  [256 x 2048] (ACT adds c via per-partition bias).
  - v = W_v x + b_v token-major [tok, e] with a ones-column appended, so the
    softmax normalizer Z drops out of the attn@v matmul for free.
  - Scores k-major: S^T[k, q] = x^T t; exp via ACT (scale=1/sqrt(E), exact
    fp32 PSUM in, bf16 out), directly the stationary operand of attn@v.
  - Softmax without max-subtraction: scores are O(+-7) for unit-scale data.
  - attn@v gives o token-major [q, e] plus Z in column 256; normalize by
    1/Z per-partition, PE-transpose 128x128 blocks to feature-major, then
    out-projection + bias per q block, DMA out in bf16.

Measured rel err vs the fp32 reference: ~2e-3 absmax-relative (fp8 x noise
through the attention path; the residual is exact f32 host-side).
"""

import contextlib

import ml_dtypes
import numpy as np

import jax
from jax.experimental.shard_map import shard_map
from jax.sharding import Mesh, NamedSharding, PartitionSpec

import concourse.bacc as bacc
import concourse.bass2jax as bass2jax
import concourse.tile as tile
from concourse import mybir
from concourse.bass_utils import run_bass_kernel_spmd

F32 = mybir.dt.float32
BF16 = mybir.dt.bfloat16
FP8 = mybir.dt.float8e4
INT8 = mybir.dt.int8
AF = mybir.ActivationFunctionType
AX = mybir.AxisListType
ALU = mybir.AluOpType

# ---------------------------------------------------------------------------
# run_bass_via_pjrt rebuilds jax.jit(shard_map(...)) from a fresh closure on
# every call, so each kernel() invocation pays a full retrace + relower
# (~120ms). Patch in a behaviorally identical variant that caches the jitted
# callable per (nc, n_cores); run_bass_kernel_spmd picks it up via the module
# attribute. Every call still ships all inputs and executes on hardware.
_ORIG_RUN_VIA_PJRT = bass2jax.run_bass_via_pjrt
_JIT_CACHE = {}
# ncs whose programs write every element of every output: for these the
# pre-zeroed output operands can be cached device-resident (no donation, no
# per-call host->device zeros upload) -- the NEFF output never depends on
# their initial contents.
_WRITES_ALL_OUTPUTS = set()


def _cached_run_bass_via_pjrt(nc, in_maps, n_cores):
    if (nc.dbg_addr is not None or n_cores == 1
            or id(nc) not in _WRITES_ALL_OUTPUTS):
        return _ORIG_RUN_VIA_PJRT(nc, in_maps, n_cores)
    key = (id(nc), n_cores)
    ent = _JIT_CACHE.get(key)
    if ent is None:
        bass2jax.install_neuronx_cc_hook()
        partition_name = (nc.partition_id_tensor.name
                          if nc.partition_id_tensor else None)
        in_names, out_names, out_avals = [], [], []
        for alloc in nc.m.functions[0].allocations:
            if not isinstance(alloc, mybir.MemoryLocationSet):
                continue
            name = alloc.memorylocations[0].name
            if alloc.kind == "ExternalInput":
                if name != partition_name:
                    in_names.append(name)
            elif alloc.kind == "ExternalOutput":
                out_names.append(name)
                out_avals.append(jax.core.ShapedArray(
                    tuple(alloc.tensor_shape), mybir.dt.np(alloc.dtype)))
        n_params, n_outs = len(in_names), len(out_names)
        all_in = tuple(in_names + out_names
                       + ([partition_name] if partition_name else []))

        def _body(*args):
            operands = list(args)
            if partition_name is not None:
                operands.append(bass2jax.partition_id_tensor())
            outs = bass2jax._bass_exec_p.bind(
                *operands, out_avals=tuple(out_avals), in_names=all_in,
                out_names=tuple(out_names), lowering_input_output_aliases=(),
                sim_require_finite=True, sim_require_nnan=True, nc=nc)
            return tuple(outs)

        mesh = Mesh(np.asarray(jax.devices()[:n_cores]), ("core",))
        sharded = jax.jit(
            shard_map(_body, mesh=mesh,
                      in_specs=(PartitionSpec("core"),) * (n_params + n_outs),
                      out_specs=(PartitionSpec("core"),) * n_outs,
                      check_rep=False),
            keep_unused=True)
        sh = NamedSharding(mesh, PartitionSpec("core"))
        zdev = [jax.device_put(
            np.zeros((n_cores * av.shape[0], *av.shape[1:]), av.dtype), sh)
            for av in out_avals]
        ent = (sharded, in_names, out_names, out_avals, zdev)
        _JIT_CACHE[key] = ent
    sharded, in_names, out_names, out_avals, zdev = ent
    override = getattr(nc, "_concat_override", None)
    if override is not None:
        concat_in = [override[nm] for nm in in_names]
    else:
        concat_in = [
            np.concatenate([np.asarray(m[nm]) for m in in_maps], axis=0)
            for nm in in_names]
    out_arrs = sharded(*concat_in, *zdev)
    # kick off all shard downloads concurrently before the blocking reads
    for o in out_arrs:
        for s in o.addressable_shards:
            s.data.copy_to_host_async()
    return [
        {nm: np.asarray(out_arrs[i]).reshape(n_cores, *out_avals[i].shape)[c]
         for i, nm in enumerate(out_names)}
        for c in range(n_cores)
    ]


bass2jax.run_bass_via_pjrt = _cached_run_bass_via_pjrt

E = 256          # embed dim
NTOK = 4096      # tokens per batch element (64*64)
NQ = 2048        # queries per core
P = 128          # partitions
NEC = 2          # e-chunks (E / P)
NKC = NTOK // P  # 32 k-chunks
QB = 512         # q block (scores free dim)
NQB = NQ // QB   # q blocks
EXP_SCALE = 1.0 / 16.0  # 1/sqrt(E)

NP_FP8 = ml_dtypes.float8_e4m3
NP_BF16 = ml_dtypes.bfloat16

N_CORES = 8


def build_nc(reps=1):
    """reps != 1 wraps the body in a HW For_i loop (used only for wall-clock
    timing via the reps-slope method; the production path is reps=1)."""
    nc = bacc.Bacc(None, target_bir_lowering=False, num_devices=N_CORES)

    # each core ships only its own query half; the peer's half (needed for
    # K/V) arrives device-to-device via a pair AllGather
    xh = nc.dram_tensor("xh", [E, NQ], FP8, kind="ExternalInput")
    # packed weights [256, 832]: [0:E]=A (t = A x_q stationary), [E:2E]=W_v^T,
    # [2E:3E]=W_o^T, [3E:3E+64]=identity (two [128,64] halves stacked).
    # Each core ships only a 1/8 column shard; an 8-rank AllGather rebuilds it.
    WCOLS = 3 * E + 64
    WSH = WCOLS // N_CORES
    wsh = nc.dram_tensor("wsh", [E, WSH], BF16, kind="ExternalInput")
    # packed per-partition biases: cols 0:2 = c (t bias), 2:4 = out bias
    bpk = nc.dram_tensor("bpk", [P, 4], F32, kind="ExternalInput")
    bv = nc.dram_tensor("bv", [E], F32, kind="ExternalInput")
    # o ships int8 with a per-(feature-row, 256-query-block) scale: same
    # precision class as bf16 at half the tunnel bytes
    out = nc.dram_tensor("out", [E, NQ], INT8, kind="ExternalOutput")
    osc = nc.dram_tensor("osc", [P, NEC, 8], F32, kind="ExternalOutput")

    with tile.TileContext(nc) as tc:
        with (
            tc.tile_pool(name="dram", bufs=1, space="DRAM") as dramp,
            tc.tile_pool(name="const", bufs=1) as const,
            tc.tile_pool(name="x8p", bufs=1) as x8p,
            tc.tile_pool(name="xpool", bufs=1) as xpool,
            tc.tile_pool(name="kqv", bufs=1) as kqv,
            tc.tile_pool(name="expp", bufs=2) as expp,
            tc.tile_pool(name="ofm", bufs=1) as ofm,
            tc.tile_pool(name="small", bufs=4) as small,
            tc.tile_pool(name="outp", bufs=3) as outp,
            tc.tile_pool(name="oscp", bufs=1) as oscp,
            tc.tile_pool(name="psA", bufs=2, space="PSUM") as psA,
            tc.tile_pool(name="psO", bufs=2, space="PSUM") as psO,
            tc.tile_pool(name="psT", bufs=2, space="PSUM") as psT,
        ):
            t = {}

            def emit_loads():
                # weights: 8-rank AllGather of the 1/8 column shards
                wg_in = dramp.tile([E, WSH], BF16, tag="wg_in", name="wg_in")
                wg = dramp.tile([N_CORES, E, WSH], BF16, tag="wg",
                                addr_space="Shared", name="wg")
                nc.sync.dma_start(out=wg_in[:, :], in_=wsh[:, :])
                nc.gpsimd.collective_compute(
                    "AllGather", ALU.bypass,
                    replica_groups=[list(range(N_CORES))],
                    ins=[wg_in[:, :]], outs=[wg[:, :, :]])
                t["wpk_sb"] = const.tile([P, NEC, WCOLS], BF16, tag="wpk",
                                         name="wpk_sb")
                for r in range(N_CORES):
                    for ec in range(NEC):
                        nc.sync.dma_start(
                            out=t["wpk_sb"][:, ec, r * WSH:(r + 1) * WSH],
                            in_=wg[r, ec * P:(ec + 1) * P, :])
                t["bpk_sb"] = const.tile([P, 4], F32, tag="bpk", name="bpk_sb")
                nc.sync.dma_start(out=t["bpk_sb"], in_=bpk[:, :])
                t["bv_bc"] = const.tile([P, E], F32, tag="bv", name="bv_bc")
                nc.sync.dma_start(out=t["bv_bc"],
                                  in_=bv[:].partition_broadcast(P))
                # identity for PE transpose: unpack the two [128, 64] halves
                t["ident_sb"] = const.tile([P, P], BF16, tag="ident",
                                           name="ident_sb")
                for ec in range(NEC):
                    nc.vector.tensor_copy(
                        t["ident_sb"][:, ec * 64:(ec + 1) * 64],
                        t["wpk_sb"][:, ec, 3 * E:3 * E + 64])
                # pair AllGather: own query half -> both halves of x[b].
                # xg is flat rank-concat, so declare it [2, E, NQ].
                xg_in = dramp.tile([E, NQ], FP8, tag="xg_in", name="xg_in")
                xg = dramp.tile([2, E, NQ], FP8, tag="xg", name="xg")
                nc.sync.dma_start(out=xg_in[:, :], in_=xh[:, :])
                nc.gpsimd.collective_compute(
                    "AllGather", ALU.bypass,
                    replica_groups=[[2 * i, 2 * i + 1]
                                    for i in range(N_CORES // 2)],
                    ins=[xg_in[:, :]], outs=[xg[:, :, :]])
                # query path straight from the input (position-independent
                # of core parity); not gated on the collective
                t["xq8_sb"] = x8p.tile([P, NEC, NQ], FP8, tag="xq8",
                                       name="xq8_sb")
                t["xq_sb"] = xpool.tile([P, NEC, NQ], BF16, tag="xq",
                                        name="xq_sb")
                for tt in range(NQ // 512):
                    for ec in range(NEC):
                        nc.sync.dma_start(
                            out=t["xq8_sb"][:, ec, tt * 512:(tt + 1) * 512],
                            in_=xh[ec * P:(ec + 1) * P,
                                   tt * 512:(tt + 1) * 512])
                for tt in range(NQ // 512):
                    for ec in range(NEC):
                        nc.vector.tensor_copy(
                            t["xq_sb"][:, ec, tt * 512:(tt + 1) * 512],
                            t["xq8_sb"][:, ec, tt * 512:(tt + 1) * 512])
                # K/V token range: both halves from the gathered buffer
                t["x8_sb"] = x8p.tile([P, NEC, NTOK], FP8, tag="x8",
                                      name="x8_sb")
                t["xb_sb"] = xpool.tile([P, NEC, NTOK], BF16, tag="xb",
                                        name="xb_sb")
                for r in range(2):
                    for tt in range(NQ // 512):
                        for ec in range(NEC):
                            nc.sync.dma_start(
                                out=t["x8_sb"][:, ec,
                                               r * NQ + tt * 512:
                                               r * NQ + (tt + 1) * 512],
                                in_=xg[r, ec * P:(ec + 1) * P,
                                       tt * 512:(tt + 1) * 512])
                for tt in range(NTOK // 512):
                    for ec in range(NEC):
                        nc.vector.tensor_copy(
                            t["xb_sb"][:, ec, tt * 512:(tt + 1) * 512],
                            t["x8_sb"][:, ec, tt * 512:(tt + 1) * 512])

            def emit_compute():
                wpk_sb, bpk_sb = t["wpk_sb"], t["bpk_sb"]
                bv_bc, ident_sb, xb_sb = t["bv_bc"], t["ident_sb"], t["xb_sb"]
                xq_sb = t["xq_sb"]

                t_sb = kqv.tile([P, NEC, NQ], BF16, tag="t", name="t_sb")
                v_sb = kqv.tile([P, NKC, E + 1], BF16, tag="v", name="v_sb")

                # ---- t = A x_q + c  (A = W_q^T W_k folded host-side; the
                # k-bias cancels in softmax and q/k are never materialized)
                for tt in range(NQ // 512):
                    for eo in range(NEC):
                        ps_full = psA.tile([P, 2, QB], F32, tag="sc",
                                           name="ps_t")
                        ps = ps_full[:, 0, :]
                        for ec in range(NEC):
                            nc.tensor.matmul(
                                ps,
                                wpk_sb[:, ec, eo * P:(eo + 1) * P],
                                xq_sb[:, ec, tt * 512:(tt + 1) * 512],
                                start=(ec == 0), stop=(ec == NEC - 1))
                        nc.scalar.activation(
                            t_sb[:, eo, tt * 512:(tt + 1) * 512], ps,
                            AF.Identity, bias=bpk_sb[:, eo:eo + 1])

                # ---- v = W_v x + b_v, token-major, ones column for Z
                for tcb in range(NKC):
                    ps_full = psA.tile([P, 2, QB], F32, tag="sc", name="ps_v")
                    ps = ps_full[:, 0, :E]
                    for ec in range(NEC):
                        nc.tensor.matmul(
                            ps,
                            xb_sb[:, ec, tcb * P:(tcb + 1) * P],
                            wpk_sb[:, ec, E:2 * E],
                            start=(ec == 0), stop=(ec == NEC - 1))
                    nc.vector.tensor_add(v_sb[:, tcb, 0:E], ps, bv_bc)
                nc.vector.memset(v_sb[:, :, E:E + 1], 1.0)

                o_fm = ofm.tile([P, NEC, NQ], BF16, tag="o_fm", name="o_fm")
                osc_sb = oscp.tile([P, NEC, 8], F32, tag="osc", name="osc_sb")

                # ---- attention, per q block
                for qb in range(NQB):
                    q0 = qb * QB
                    expS = expp.tile([P, NKC, QB], BF16, tag="expS",
                                     name="expS")
                    for kcg in range(NKC // 2):
                        ps = psA.tile([P, 2, QB], F32, tag="sc", name="ps_s")
                        for kk in range(2):
                            kc = kcg * 2 + kk
                            for ec in range(NEC):
                                nc.tensor.matmul(
                                    ps[:, kk, :],
                                    xb_sb[:, ec, kc * P:(kc + 1) * P],
                                    t_sb[:, ec, q0:q0 + QB],
                                    start=(ec == 0), stop=(ec == NEC - 1))
                        nc.scalar.activation(
                            expS[:, kcg * 2:(kcg + 1) * 2, :], ps, AF.Exp,
                            scale=EXP_SCALE)
                    for qq in range(QB // P):
                        po = psO.tile([P, E + 1], F32, tag="po", name="po")
                        for kc in range(NKC):
                            nc.tensor.matmul(
                                po,
                                expS[:, kc, qq * P:(qq + 1) * P],
                                v_sb[:, kc, :],
                                start=(kc == 0), stop=(kc == NKC - 1))
                        zr = small.tile([P, 1], F32, tag="zr", name="zr")
                        nc.vector.reciprocal(zr, po[:, E:E + 1])
                        o_tm = small.tile([P, E], BF16, tag="o_tm",
                                          name="o_tm")
                        nc.vector.tensor_scalar_mul(o_tm, po[:, 0:E], zr)
                        for ec in range(NEC):
                            pt = psT.tile([P, P], BF16, tag="pt", name="pt")
                            nc.tensor.transpose(
                                pt, o_tm[:, ec * P:(ec + 1) * P], ident_sb)
                            nc.vector.tensor_copy(
                                o_fm[:, ec, q0 + qq * P:q0 + (qq + 1) * P], pt)

                    # out projection + bias for this q block (residual is
                    # added host-side in f32); quantize to int8 with a
                    # per-(row, block) scale = absmax/127
                    for fc in range(NEC):
                        for qh in range(QB // 256):
                            blk = qb * 2 + qh
                            pso = psO.tile([P, E + 1], F32, tag="po",
                                           name="pso")
                            ps2 = pso[:, 0:256]
                            for ec in range(NEC):
                                nc.tensor.matmul(
                                    ps2,
                                    wpk_sb[:, ec,
                                           2 * E + fc * P:2 * E + (fc + 1) * P],
                                    o_fm[:, ec,
                                         q0 + qh * 256:q0 + (qh + 1) * 256],
                                    start=(ec == 0), stop=(ec == NEC - 1))
                            t2 = outp.tile([P, 256], F32, tag="t2", name="t2")
                            nc.vector.tensor_scalar_add(
                                t2, ps2, bpk_sb[:, 2 + fc:3 + fc])
                            rmax = small.tile([P, 1], F32, tag="rmax",
                                              name="rmax")
                            nc.vector.tensor_reduce(
                                rmax, t2, axis=AX.X, op=ALU.max,
                                apply_absolute_value=True)
                            nc.scalar.activation(
                                osc_sb[:, fc, blk:blk + 1], rmax,
                                AF.Identity, scale=1.0 / 127.0)
                            inv = small.tile([P, 1], F32, tag="inv",
                                             name="inv")
                            nc.vector.reciprocal(
                                inv, osc_sb[:, fc, blk:blk + 1])
                            t8 = outp.tile([P, 256], INT8, tag="t8",
                                           name="t8")
                            nc.vector.tensor_scalar_mul(t8, t2, inv)
                            nc.sync.dma_start(
                                out=out[fc * P:(fc + 1) * P,
                                        q0 + qh * 256:q0 + (qh + 1) * 256],
                                in_=t8)
                nc.sync.dma_start(out=osc[:, :, :], in_=osc_sb[:, :, :])

            loop_ctx = (tc.For_i(0, reps, 1) if reps != 1
                        else contextlib.nullcontext())
            with loop_ctx:
                emit_loads()
                emit_compute()

    nc.compile()
    return nc


_NC = {}


def _get_nc(reps=1):
    if reps not in _NC:
        _NC[reps] = build_nc(reps)
        if reps != 0:
            # the program writes every element of out/osc, so the cached
            # device-resident zero operands path is safe
            _WRITES_ALL_OUTPUTS.add(id(_NC[reps]))
    return _NC[reps]


def make_in_maps(x, qkv_w, qkv_b, out_w, out_b):
    b, e, h, w = x.shape
    n = h * w
    qkv_w = np.asarray(qkv_w).astype(np.float32)
    qkv_b = np.asarray(qkv_b).astype(np.float32)
    out_w = np.asarray(out_w).astype(np.float32)
    out_b = np.asarray(out_b).astype(np.float32)
    xf = np.ascontiguousarray(np.asarray(x, dtype=np.float32).reshape(b, e, n))
    x8 = xf.astype(NP_FP8)
    wq, wk, wv = qkv_w[:E], qkv_w[E:2 * E], qkv_w[2 * E:]
    # t = A x_q + c reproduces q.k up to the k-bias (constant over keys,
    # cancels in softmax): stationary A[e2, e1] = sum_f wq[f,e2] wk[f,e1]
    A = wq.T @ wk
    c = wk.T @ qkv_b[:E]
    ident = np.eye(P, dtype=np.float32)
    ident_pack = np.concatenate([ident[:, :64], ident[:, 64:]], axis=0)
    wpk = np.ascontiguousarray(np.concatenate(
        [A, wv.T, out_w.T, ident_pack], axis=1)).astype(NP_BF16)
    WSH = wpk.shape[1] // N_CORES
    bpk = np.ascontiguousarray(
        np.stack([c[:P], c[P:], out_b[:P], out_b[P:]], axis=1)
    ).astype(np.float32)
    bv_a = np.ascontiguousarray(qkv_b[2 * E:])
    in_maps = []
    for core in range(N_CORES):
        bi, half = divmod(core, 2)
        in_maps.append({
            "xh": np.ascontiguousarray(x8[bi][:, half * NQ:(half + 1) * NQ]),
            "wsh": np.ascontiguousarray(wpk[:, core * WSH:(core + 1) * WSH]),
            "bpk": bpk, "bv": bv_a,
        })
    return in_maps


def assemble(results, x, xf=None):
    b, e, h, w = x.shape
    n = h * w
    if xf is None:
        xf = np.asarray(x, dtype=np.float32).reshape(b, e, n)
    out = np.empty((b, e, n), np.float32)
    for core in range(N_CORES):
        bi, half = divmod(core, 2)
        sl = slice(half * NQ, (half + 1) * NQ)
        o = np.asarray(results[core]["out"]).astype(np.float32)
        sc = np.asarray(results[core]["osc"])  # [P, NEC, 8] = rows p, fc, blk
        o = o.reshape(NEC, P, 8, 256) * sc.transpose(1, 0, 2)[:, :, :, None]
        out[bi][:, sl] = xf[bi][:, sl] + o.reshape(E, NQ)
    return out.reshape(b, e, h, w)


def kernel(x, qkv_w, qkv_b, out_w, out_b):
    b, e, h, w = x.shape
    n = h * w
    qkv_w = np.asarray(qkv_w).astype(np.float32)
    qkv_b = np.asarray(qkv_b).astype(np.float32)
    out_w = np.asarray(out_w).astype(np.float32)
    out_b = np.asarray(out_b).astype(np.float32)
    xf = np.asarray(x, dtype=np.float32).reshape(b, e, n)
    # convert each core's x shard to fp8 and device_put it immediately
    # (async) so upload pipelines with the remaining host-side prep
    devs = jax.devices()[:N_CORES]
    mesh = Mesh(np.asarray(devs), ("core",))
    xsh_np = []
    xsh_dev = []
    for core in range(N_CORES):
        bi, half = divmod(core, 2)
        xs = xf[bi][:, half * NQ:(half + 1) * NQ].astype(NP_FP8)
        xsh_np.append(xs)
        xsh_dev.append(jax.device_put(xs, devs[core]))
    x_g = jax.make_array_from_single_device_arrays(
        (N_CORES * E, NQ), NamedSharding(mesh, PartitionSpec("core")),
        xsh_dev)
    wq, wk, wv = qkv_w[:E], qkv_w[E:2 * E], qkv_w[2 * E:]
    A = wq.T @ wk
    c = wk.T @ qkv_b[:E]
    ident = np.eye(P, dtype=np.float32)
    ident_pack = np.concatenate([ident[:, :64], ident[:, 64:]], axis=0)
    wpk = np.concatenate([A, wv.T, out_w.T, ident_pack], axis=1).astype(
        NP_BF16)
    WSH = wpk.shape[1] // N_CORES
    wsh_g = np.ascontiguousarray(
        wpk.reshape(E, N_CORES, WSH).transpose(1, 0, 2).reshape(
            N_CORES * E, WSH))
    bpk = np.stack([c[:P], c[P:], out_b[:P], out_b[P:]], axis=1).astype(
        np.float32)
    bpk_g = np.tile(bpk, (N_CORES, 1))
    bv_g = np.tile(np.ascontiguousarray(qkv_b[2 * E:]), N_CORES)
    in_maps = [
        {"xh": xsh_np[core],
         "wsh": wsh_g[core * E:(core + 1) * E],
         "bpk": bpk_g[core * P:(core + 1) * P],
         "bv": bv_g[core * E:(core + 1) * E]}
        for core in range(N_CORES)
    ]
    nc = _get_nc()
    nc._concat_override = {"xh": x_g, "wsh": wsh_g, "bpk": bpk_g, "bv": bv_g}
    try:
        res = run_bass_kernel_spmd(nc, in_maps, core_ids=list(range(N_CORES)))
    finally:
        nc._concat_override = None
    return assemble(res.results, x, xf=xf)


# revision 35
# speedup vs baseline: 1.0810x; 1.0098x over previous
"""AttentionBlock Trainium2 kernel (self-contained).

Problem: x[4,256,64,64] -> qkv 1x1 conv -> single-head self-attention over
the 4096 spatial tokens -> out 1x1 conv -> residual.

Under the axon relay the end-to-end time of kernel() is dominated by
host<->device transfer over the tunnel (~56 MB/s up, ~30 MB/s down measured,
transfers serialize globally), not on-device compute, so the design
minimizes shipped bytes and per-call dispatch overhead:

  - Each core uploads ONLY its own 2048-query half of x, in fp8 (e4m3,
    0.5 MB): the attention path tolerates fp8 activation noise, and the f32
    residual (x + o) is added on the HOST, so the device never needs a
    precise x. The peer half (needed for K/V) arrives device-to-device via
    a pair AllGather -- x never ships twice over the tunnel.
  - The q/k weights are folded on the host: the k-side bias cancels in
    softmax and scores only need t = A x_q + c with A = W_q^T W_k (256x256)
    and c = W_k^T b_q, so W_q/W_k never ship -- just A (bf16). Together with
    W_v^T, W_o^T and the transpose identity, all weights form ONE packed
    bf16 tensor [256, 832], of which each core uploads a 1/8 column shard;
    an 8-rank AllGather rebuilds the full copy on every core.
  - Output is the projected o WITHOUT the residual, quantized to int8 with
    per-(feature-row, 256-query-block) scales (absmax/127, a second tiny f32
    output): same error class as bf16 at half the download bytes. The host
    dequantizes and adds x in f32.
  - run_bass_via_pjrt is patched with a behaviorally identical variant that
    caches the jitted callable (the library retraces per call), keeps the
    pre-zeroed output operands device-resident (this program overwrites
    every output element, so donation + per-call zero upload is
    unnecessary), and pipelines shard uploads/downloads with
    device_put/copy_to_host_async.

Sharding: 8 cores = 4 batch elements x 2 query halves. Each core handles one
batch element's full K/V token range (4096, gathered) and its own 2048
queries, flash-style on-chip: the [4096 x 2048] score matrix never touches
HBM. Key order inside the softmax sum is irrelevant, so the gathered
natural-order buffer feeds K/V directly on both pair members.

Per-core dataflow (feature-major x[b] reshaped [256, 4096], fp8):
  - upconvert x8 -> bf16 once in SBUF; all matmuls run bf16 with f32 PSUM.
  - t = A x_q + c  [256 x 2048] (ACT adds c via per-partition bias).
  - v = W_v x + b_v token-major [tok, e] with a ones-column appended, so the
    softmax normalizer Z drops out of the attn@v matmul for free.
  - Scores k-major: S^T[k, q] = x^T t; exp via ACT (scale=1/sqrt(E), exact
    fp32 PSUM in, bf16 out), directly the stationary operand of attn@v.
  - Softmax without max-subtraction: scores are O(+-7) for unit-scale data.
  - attn@v gives o token-major [q, e] plus Z in column 256; normalize by
    1/Z per-partition, PE-transpose 128x128 blocks to feature-major, then
    out-projection + bias per q block, DMA out in bf16.

Measured rel err vs the fp32 reference: ~2e-3 absmax-relative (fp8 x noise
through the attention path; the residual is exact f32 host-side).
"""

import contextlib

import ml_dtypes
import numpy as np

import jax
from jax.experimental.shard_map import shard_map
from jax.sharding import Mesh, NamedSharding, PartitionSpec

import concourse.bacc as bacc
import concourse.bass2jax as bass2jax
import concourse.tile as tile
from concourse import mybir
from concourse.bass_utils import run_bass_kernel_spmd

F32 = mybir.dt.float32
BF16 = mybir.dt.bfloat16
FP8 = mybir.dt.float8e4
INT8 = mybir.dt.int8
AF = mybir.ActivationFunctionType
AX = mybir.AxisListType
ALU = mybir.AluOpType

# ---------------------------------------------------------------------------
# run_bass_via_pjrt rebuilds jax.jit(shard_map(...)) from a fresh closure on
# every call, so each kernel() invocation pays a full retrace + relower
# (~120ms). Patch in a behaviorally identical variant that caches the jitted
# callable per (nc, n_cores); run_bass_kernel_spmd picks it up via the module
# attribute. Every call still ships all inputs and executes on hardware.
_ORIG_RUN_VIA_PJRT = bass2jax.run_bass_via_pjrt
_JIT_CACHE = {}
# ncs whose programs write every element of every output: for these the
# pre-zeroed output operands can be cached device-resident (no donation, no
# per-call host->device zeros upload) -- the NEFF output never depends on
# their initial contents.
_WRITES_ALL_OUTPUTS = set()


def _cached_run_bass_via_pjrt(nc, in_maps, n_cores):
    if (nc.dbg_addr is not None or n_cores == 1
            or id(nc) not in _WRITES_ALL_OUTPUTS):
        return _ORIG_RUN_VIA_PJRT(nc, in_maps, n_cores)
    key = (id(nc), n_cores)
    ent = _JIT_CACHE.get(key)
    if ent is None:
        bass2jax.install_neuronx_cc_hook()
        partition_name = (nc.partition_id_tensor.name
                          if nc.partition_id_tensor else None)
        in_names, out_names, out_avals = [], [], []
        for alloc in nc.m.functions[0].allocations:
            if not isinstance(alloc, mybir.MemoryLocationSet):
                continue
            name = alloc.memorylocations[0].name
            if alloc.kind == "ExternalInput":
                if name != partition_name:
                    in_names.append(name)
            elif alloc.kind == "ExternalOutput":
                out_names.append(name)
                out_avals.append(jax.core.ShapedArray(
                    tuple(alloc.tensor_shape), mybir.dt.np(alloc.dtype)))
        n_params, n_outs = len(in_names), len(out_names)
        all_in = tuple(in_names + out_names
                       + ([partition_name] if partition_name else []))

        def _body(*args):
            operands = list(args)
            if partition_name is not None:
                operands.append(bass2jax.partition_id_tensor())
            outs = bass2jax._bass_exec_p.bind(
                *operands, out_avals=tuple(out_avals), in_names=all_in,
                out_names=tuple(out_names), lowering_input_output_aliases=(),
                sim_require_finite=True, sim_require_nnan=True, nc=nc)
            return tuple(outs)

        mesh = Mesh(np.asarray(jax.devices()[:n_cores]), ("core",))
        sharded = jax.jit(
            shard_map(_body, mesh=mesh,
                      in_specs=(PartitionSpec("core"),) * (n_params + n_outs),
                      out_specs=(PartitionSpec("core"),) * n_outs,
                      check_rep=False),
            keep_unused=True)
        sh = NamedSharding(mesh, PartitionSpec("core"))
        zdev = [jax.device_put(
            np.zeros((n_cores * av.shape[0], *av.shape[1:]), av.dtype), sh)
            for av in out_avals]
        ent = (sharded, in_names, out_names, out_avals, zdev)
        _JIT_CACHE[key] = ent
    sharded, in_names, out_names, out_avals, zdev = ent
    override = getattr(nc, "_concat_override", None)
    if override is not None:
        concat_in = [override[nm] for nm in in_names]
    else:
        concat_in = [
            np.concatenate([np.asarray(m[nm]) for m in in_maps], axis=0)
            for nm in in_names]
    out_arrs = sharded(*concat_in, *zdev)
    # kick off all shard downloads concurrently before the blocking reads
    for o in out_arrs:
        for s in o.addressable_shards:
            s.data.copy_to_host_async()
    return [
        {nm: np.asarray(out_arrs[i]).reshape(n_cores, *out_avals[i].shape)[c]
         for i, nm in enumerate(out_names)}
        for c in range(n_cores)
    ]


bass2jax.run_bass_via_pjrt = _cached_run_bass_via_pjrt

E = 256          # embed dim
NTOK = 4096      # tokens per batch element (64*64)
NQ = 2048        # queries per core
P = 128          # partitions
NEC = 2          # e-chunks (E / P)
NKC = NTOK // P  # 32 k-chunks
QB = 512         # q block (scores free dim)
NQB = NQ // QB   # q blocks
EXP_SCALE = 1.0 / 16.0  # 1/sqrt(E)

NP_FP8 = ml_dtypes.float8_e4m3
NP_BF16 = ml_dtypes.bfloat16

N_CORES = 8


def build_nc(reps=1):
    """reps != 1 wraps the body in a HW For_i loop (used only for wall-clock
    timing via the reps-slope method; the production path is reps=1)."""
    nc = bacc.Bacc(None, target_bir_lowering=False, num_devices=N_CORES)

    # each core ships only its own query half; the peer's half (needed for
    # K/V) arrives device-to-device via a pair AllGather
    xh = nc.dram_tensor("xh", [E, NQ], FP8, kind="ExternalInput")
    # packed weights [256, 832]: [0:E]=A (t = A x_q stationary), [E:2E]=W_v^T,
    # [2E:3E]=W_o^T, [3E:3E+64]=identity (two [128,64] halves stacked).
    # Each core ships only a 1/8 column shard; an 8-rank AllGather rebuilds it.
    WCOLS = 3 * E + 64
    WSH = WCOLS // N_CORES
    wsh = nc.dram_tensor("wsh", [E, WSH], BF16, kind="ExternalInput")
    # packed per-partition biases: cols 0:2 = c (t bias), 2:4 = out bias
    bpk = nc.dram_tensor("bpk", [P, 4], F32, kind="ExternalInput")
    bv = nc.dram_tensor("bv", [E], F32, kind="ExternalInput")
    # o ships int8 with a per-(feature-row, 256-query-block) scale: same
    # precision class as bf16 at half the tunnel bytes
    out = nc.dram_tensor("out", [E, NQ], INT8, kind="ExternalOutput")
    osc = nc.dram_tensor("osc", [P, NEC, 8], F32, kind="ExternalOutput")

    with tile.TileContext(nc) as tc:
        with (
            tc.tile_pool(name="dram", bufs=1, space="DRAM") as dramp,
            tc.tile_pool(name="const", bufs=1) as const,
            tc.tile_pool(name="x8p", bufs=1) as x8p,
            tc.tile_pool(name="xpool", bufs=1) as xpool,
            tc.tile_pool(name="kqv", bufs=1) as kqv,
            tc.tile_pool(name="expp", bufs=2) as expp,
            tc.tile_pool(name="ofm", bufs=1) as ofm,
            tc.tile_pool(name="small", bufs=4) as small,
            tc.tile_pool(name="outp", bufs=3) as outp,
            tc.tile_pool(name="oscp", bufs=1) as oscp,
            tc.tile_pool(name="psA", bufs=2, space="PSUM") as psA,
            tc.tile_pool(name="psO", bufs=2, space="PSUM") as psO,
            tc.tile_pool(name="psT", bufs=2, space="PSUM") as psT,
        ):
            t = {}

            def emit_loads():
                # weights: 8-rank AllGather of the 1/8 column shards
                wg_in = dramp.tile([E, WSH], BF16, tag="wg_in", name="wg_in")
                wg = dramp.tile([N_CORES, E, WSH], BF16, tag="wg",
                                addr_space="Shared", name="wg")
                nc.sync.dma_start(out=wg_in[:, :], in_=wsh[:, :])
                nc.gpsimd.collective_compute(
                    "AllGather", ALU.bypass,
                    replica_groups=[list(range(N_CORES))],
                    ins=[wg_in[:, :]], outs=[wg[:, :, :]])
                t["wpk_sb"] = const.tile([P, NEC, WCOLS], BF16, tag="wpk",
                                         name="wpk_sb")
                for r in range(N_CORES):
                    for ec in range(NEC):
                        nc.sync.dma_start(
                            out=t["wpk_sb"][:, ec, r * WSH:(r + 1) * WSH],
                            in_=wg[r, ec * P:(ec + 1) * P, :])
                t["bpk_sb"] = const.tile([P, 4], F32, tag="bpk", name="bpk_sb")
                nc.sync.dma_start(out=t["bpk_sb"], in_=bpk[:, :])
                t["bv_bc"] = const.tile([P, E], F32, tag="bv", name="bv_bc")
                nc.sync.dma_start(out=t["bv_bc"],
                                  in_=bv[:].partition_broadcast(P))
                # identity for PE transpose: unpack the two [128, 64] halves
                t["ident_sb"] = const.tile([P, P], BF16, tag="ident",
                                           name="ident_sb")
                for ec in range(NEC):
                    nc.vector.tensor_copy(
                        t["ident_sb"][:, ec * 64:(ec + 1) * 64],
                        t["wpk_sb"][:, ec, 3 * E:3 * E + 64])
                # pair AllGather: own query half -> both halves of x[b].
                # xg is flat rank-concat, so declare it [2, E, NQ].
                xg_in = dramp.tile([E, NQ], FP8, tag="xg_in", name="xg_in")
                xg = dramp.tile([2, E, NQ], FP8, tag="xg", name="xg")
                nc.sync.dma_start(out=xg_in[:, :], in_=xh[:, :])
                nc.gpsimd.collective_compute(
                    "AllGather", ALU.bypass,
                    replica_groups=[[2 * i, 2 * i + 1]
                                    for i in range(N_CORES // 2)],
                    ins=[xg_in[:, :]], outs=[xg[:, :, :]])
                # query path straight from the input (position-independent
                # of core parity); not gated on the collective
                t["xq8_sb"] = x8p.tile([P, NEC, NQ], FP8, tag="xq8",
                                       name="xq8_sb")
                t["xq_sb"] = xpool.tile([P, NEC, NQ], BF16, tag="xq",
                                        name="xq_sb")
                for tt in range(NQ // 512):
                    for ec in range(NEC):
                        nc.sync.dma_start(
                            out=t["xq8_sb"][:, ec, tt * 512:(tt + 1) * 512],
                            in_=xh[ec * P:(ec + 1) * P,
                                   tt * 512:(tt + 1) * 512])
                for tt in range(NQ // 512):
                    for ec in range(NEC):
                        nc.vector.tensor_copy(
                            t["xq_sb"][:, ec, tt * 512:(tt + 1) * 512],
                            t["xq8_sb"][:, ec, tt * 512:(tt + 1) * 512])
                # K/V token range: both halves from the gathered buffer
                t["x8_sb"] = x8p.tile([P, NEC, NTOK], FP8, tag="x8",
                                      name="x8_sb")
                t["xb_sb"] = xpool.tile([P, NEC, NTOK], BF16, tag="xb",
                                        name="xb_sb")
                for r in range(2):
                    for tt in range(NQ // 512):
                        for ec in range(NEC):
                            nc.sync.dma_start(
                                out=t["x8_sb"][:, ec,
                                               r * NQ + tt * 512:
                                               r * NQ + (tt + 1) * 512],
                                in_=xg[r, ec * P:(ec + 1) * P,
                                       tt * 512:(tt + 1) * 512])
                for tt in range(NTOK // 512):
                    for ec in range(NEC):
                        nc.vector.tensor_copy(
                            t["xb_sb"][:, ec, tt * 512:(tt + 1) * 512],
                            t["x8_sb"][:, ec, tt * 512:(tt + 1) * 512])

            def emit_compute():
                wpk_sb, bpk_sb = t["wpk_sb"], t["bpk_sb"]
                bv_bc, ident_sb, xb_sb = t["bv_bc"], t["ident_sb"], t["xb_sb"]
                xq_sb = t["xq_sb"]

                t_sb = kqv.tile([P, NEC, NQ], BF16, tag="t", name="t_sb")
                v_sb = kqv.tile([P, NKC, E + 1], BF16, tag="v", name="v_sb")

                # ---- t = A x_q + c  (A = W_q^T W_k folded host-side; the
                # k-bias cancels in softmax and q/k are never materialized)
                for tt in range(NQ // 512):
                    for eo in range(NEC):
                        ps_full = psA.tile([P, 2, QB], F32, tag="sc",
                                           name="ps_t")
                        ps = ps_full[:, 0, :]
                        for ec in range(NEC):
                            nc.tensor.matmul(
                                ps,
                                wpk_sb[:, ec, eo * P:(eo + 1) * P],
                                xq_sb[:, ec, tt * 512:(tt + 1) * 512],
                                start=(ec == 0), stop=(ec == NEC - 1))
                        nc.scalar.activation(
                            t_sb[:, eo, tt * 512:(tt + 1) * 512], ps,
                            AF.Identity, bias=bpk_sb[:, eo:eo + 1])

                # ---- v = W_v x + b_v, token-major, ones column for Z
                for tcb in range(NKC):
                    ps_full = psA.tile([P, 2, QB], F32, tag="sc", name="ps_v")
                    ps = ps_full[:, 0, :E]
                    for ec in range(NEC):
                        nc.tensor.matmul(
                            ps,
                            xb_sb[:, ec, tcb * P:(tcb + 1) * P],
                            wpk_sb[:, ec, E:2 * E],
                            start=(ec == 0), stop=(ec == NEC - 1))
                    nc.vector.tensor_add(v_sb[:, tcb, 0:E], ps, bv_bc)
                nc.vector.memset(v_sb[:, :, E:E + 1], 1.0)

                o_fm = ofm.tile([P, NEC, NQ], BF16, tag="o_fm", name="o_fm")
                osc_sb = oscp.tile([P, NEC, 8], F32, tag="osc", name="osc_sb")

                # ---- attention, per q block
                for qb in range(NQB):
                    q0 = qb * QB
                    expS = expp.tile([P, NKC, QB], BF16, tag="expS",
                                     name="expS")
                    for kcg in range(NKC // 2):
                        ps = psA.tile([P, 2, QB], F32, tag="sc", name="ps_s")
                        for kk in range(2):
                            kc = kcg * 2 + kk
                            for ec in range(NEC):
                                nc.tensor.matmul(
                                    ps[:, kk, :],
                                    xb_sb[:, ec, kc * P:(kc + 1) * P],
                                    t_sb[:, ec, q0:q0 + QB],
                                    start=(ec == 0), stop=(ec == NEC - 1))
                        nc.scalar.activation(
                            expS[:, kcg * 2:(kcg + 1) * 2, :], ps, AF.Exp,
                            scale=EXP_SCALE)
                    for qq in range(QB // P):
                        po = psO.tile([P, E + 1], F32, tag="po", name="po")
                        for kc in range(NKC):
                            nc.tensor.matmul(
                                po,
                                expS[:, kc, qq * P:(qq + 1) * P],
                                v_sb[:, kc, :],
                                start=(kc == 0), stop=(kc == NKC - 1))
                        zr = small.tile([P, 1], F32, tag="zr", name="zr")
                        nc.vector.reciprocal(zr, po[:, E:E + 1])
                        o_tm = small.tile([P, E], BF16, tag="o_tm",
                                          name="o_tm")
                        nc.vector.tensor_scalar_mul(o_tm, po[:, 0:E], zr)
                        for ec in range(NEC):
                            pt = psT.tile([P, P], BF16, tag="pt", name="pt")
                            nc.tensor.transpose(
                                pt, o_tm[:, ec * P:(ec + 1) * P], ident_sb)
                            nc.vector.tensor_copy(
                                o_fm[:, ec, q0 + qq * P:q0 + (qq + 1) * P], pt)

                    # out projection + bias for this q block (residual is
                    # added host-side in f32); quantize to int8 with a
                    # per-(row, block) scale = absmax/127
                    for fc in range(NEC):
                        for qh in range(QB // 256):
                            blk = qb * 2 + qh
                            pso = psO.tile([P, E + 1], F32, tag="po",
                                           name="pso")
                            ps2 = pso[:, 0:256]
                            for ec in range(NEC):
                                nc.tensor.matmul(
                                    ps2,
                                    wpk_sb[:, ec,
                                           2 * E + fc * P:2 * E + (fc + 1) * P],
                                    o_fm[:, ec,
                                         q0 + qh * 256:q0 + (qh + 1) * 256],
                                    start=(ec == 0), stop=(ec == NEC - 1))
                            t2 = outp.tile([P, 256], F32, tag="t2", name="t2")
                            nc.vector.tensor_scalar_add(
                                t2, ps2, bpk_sb[:, 2 + fc:3 + fc])
                            rmax = small.tile([P, 1], F32, tag="rmax",
                                              name="rmax")
                            nc.vector.tensor_reduce(
                                rmax, t2, axis=AX.X, op=ALU.max,
                                apply_absolute_value=True)
                            nc.scalar.activation(
                                osc_sb[:, fc, blk:blk + 1], rmax,
                                AF.Identity, scale=1.0 / 127.0)
                            inv = small.tile([P, 1], F32, tag="inv",
                                             name="inv")
                            nc.vector.reciprocal(
                                inv, osc_sb[:, fc, blk:blk + 1])
                            t8 = outp.tile([P, 256], INT8, tag="t8",
                                           name="t8")
                            nc.vector.tensor_scalar_mul(t8, t2, inv)
                            nc.sync.dma_start(
                                out=out[fc * P:(fc + 1) * P,
                                        q0 + qh * 256:q0 + (qh + 1) * 256],
                                in_=t8)
                nc.sync.dma_start(out=osc[:, :, :], in_=osc_sb[:, :, :])

            loop_ctx = (tc.For_i(0, reps, 1) if reps != 1
                        else contextlib.nullcontext())
            with loop_ctx:
                emit_loads()
                emit_compute()

    nc.compile()
    return nc


_NC = {}


def _get_nc(reps=1):
    if reps not in _NC:
        _NC[reps] = build_nc(reps)
        if reps != 0:
            # the program writes every element of out/osc, so the cached
            # device-resident zero operands path is safe
            _WRITES_ALL_OUTPUTS.add(id(_NC[reps]))
    return _NC[reps]


def make_in_maps(x, qkv_w, qkv_b, out_w, out_b):
    b, e, h, w = x.shape
    n = h * w
    qkv_w = np.asarray(qkv_w).astype(np.float32)
    qkv_b = np.asarray(qkv_b).astype(np.float32)
    out_w = np.asarray(out_w).astype(np.float32)
    out_b = np.asarray(out_b).astype(np.float32)
    xf = np.ascontiguousarray(np.asarray(x, dtype=np.float32).reshape(b, e, n))
    x8 = xf.astype(NP_FP8)
    wq, wk, wv = qkv_w[:E], qkv_w[E:2 * E], qkv_w[2 * E:]
    # t = A x_q + c reproduces q.k up to the k-bias (constant over keys,
    # cancels in softmax): stationary A[e2, e1] = sum_f wq[f,e2] wk[f,e1]
    A = wq.T @ wk
    c = wk.T @ qkv_b[:E]
    ident = np.eye(P, dtype=np.float32)
    ident_pack = np.concatenate([ident[:, :64], ident[:, 64:]], axis=0)
    wpk = np.ascontiguousarray(np.concatenate(
        [A, wv.T, out_w.T, ident_pack], axis=1)).astype(NP_BF16)
    WSH = wpk.shape[1] // N_CORES
    bpk = np.ascontiguousarray(
        np.stack([c[:P], c[P:], out_b[:P], out_b[P:]], axis=1)
    ).astype(np.float32)
    bv_a = np.ascontiguousarray(qkv_b[2 * E:])
    in_maps = []
    for core in range(N_CORES):
        bi, half = divmod(core, 2)
        in_maps.append({
            "xh": np.ascontiguousarray(x8[bi][:, half * NQ:(half + 1) * NQ]),
            "wsh": np.ascontiguousarray(wpk[:, core * WSH:(core + 1) * WSH]),
            "bpk": bpk, "bv": bv_a,
        })
    return in_maps


def assemble(results, x, xf=None):
    b, e, h, w = x.shape
    n = h * w
    if xf is None:
        xf = np.asarray(x, dtype=np.float32).reshape(b, e, n)
    out = np.empty((b, e, n), np.float32)
    for core in range(N_CORES):
        bi, half = divmod(core, 2)
        sl = slice(half * NQ, (half + 1) * NQ)
        o = np.asarray(results[core]["out"]).astype(np.float32)
        sc = np.asarray(results[core]["osc"])  # [P, NEC, 8] = rows p, fc, blk
        o = o.reshape(NEC, P, 8, 256) * sc.transpose(1, 0, 2)[:, :, :, None]
        out[bi][:, sl] = xf[bi][:, sl] + o.reshape(E, NQ)
    return out.reshape(b, e, h, w)


def kernel(x, qkv_w, qkv_b, out_w, out_b):
    b, e, h, w = x.shape
    n = h * w
    qkv_w = np.asarray(qkv_w).astype(np.float32)
    qkv_b = np.asarray(qkv_b).astype(np.float32)
    out_w = np.asarray(out_w).astype(np.float32)
    out_b = np.asarray(out_b).astype(np.float32)
    xf = np.asarray(x, dtype=np.float32).reshape(b, e, n)
    # convert each core's x shard to fp8 and device_put it immediately
    # (async) so upload pipelines with the remaining host-side prep
    devs = jax.devices()[:N_CORES]
    mesh = Mesh(np.asarray(devs), ("core",))
    xsh_np = []
    xsh_dev = []
    for core in range(N_CORES):
        bi, half = divmod(core, 2)
        xs = xf[bi][:, half * NQ:(half + 1) * NQ].astype(NP_FP8)
        xsh_np.append(xs)
        xsh_dev.append(jax.device_put(xs, devs[core]))
    x_g = jax.make_array_from_single_device_arrays(
        (N_CORES * E, NQ), NamedSharding(mesh, PartitionSpec("core")),
        xsh_dev)
    wq, wk, wv = qkv_w[:E], qkv_w[E:2 * E], qkv_w[2 * E:]
    A = wq.T @ wk
    c = wk.T @ qkv_b[:E]
    ident = np.eye(P, dtype=np.float32)
    ident_pack = np.concatenate([ident[:, :64], ident[:, 64:]], axis=0)
    wpk = np.concatenate([A, wv.T, out_w.T, ident_pack], axis=1).astype(
        NP_BF16)
    WSH = wpk.shape[1] // N_CORES
    wsh_g = np.ascontiguousarray(
        wpk.reshape(E, N_CORES, WSH).transpose(1, 0, 2).reshape(
            N_CORES * E, WSH))
    bpk = np.stack([c[:P], c[P:], out_b[:P], out_b[P:]], axis=1).astype(
        np.float32)
    bpk_g = np.tile(bpk, (N_CORES, 1))
    bv_g = np.tile(np.ascontiguousarray(qkv_b[2 * E:]), N_CORES)
    in_maps = [
        {"xh": xsh_np[core],
         "wsh": wsh_g[core * E:(core + 1) * E],
         "bpk": bpk_g[core * P:(core + 1) * P],
         "bv": bv_g[core * E:(core + 1) * E]}
        for core in range(N_CORES)
    ]
    nc = _get_nc()
    nc._concat_override = {"xh": x_g, "wsh": wsh_g, "bpk": bpk_g, "bv": bv_g}
    try:
        res = run_bass_kernel_spmd(nc, in_maps, core_ids=list(range(N_CORES)))
    finally:
        nc._concat_override = None
    return assemble(res.results, x, xf=xf)
